# revision 49
# baseline (speedup 1.0000x reference)
"""OCS fused kernel for Trainium2, data-parallel over batch across 8 cores.

Algebraic restructuring (verified vs reference to ~1e-6 in fp64):

Spatial branch (4 scan orders, shared weights) collapses to a symmetric
5-point stencil with scan-order wrap rules, and the two 1x1 convs fold
through it:
    W_proj @ y_sp = A2 @ sx + (B3 - W_proj) @ x
    sx = sum of 4 flat shifts of x (+/-1 row-major, +/-w) + col-scan wraps
Channel branch: m = g g^T is rank-1 and g = sum(x) depends only on the
input, so the HOST builds the three [128,32] rank-1 weights
MP/MQ/MR = u (x) P/Q/R exactly; the device runs three shifted matmuls,
a silu, and one [128,32] matmul.
Diff branch: |x - nb| terms are shared between opposite directions, so one
|dx| array per axis + shifted adds gives the 4-neighbor abs-diff sum S;
W_proj folds in as W_d @ S.
BatchNorm: per-core partial (sum, sumsq) in two pieces -> two 1KB
AllReduces (the first mid-run absorbs the cross-core skew so the final
one is short) -> affine apply.

Pipeline layout: x loads ride BOTH hardware DMA queues (even windows on
sync, odd on scalar); per 2048-col group the DVE window ops, the PE
channel matmuls and the PE main matmuls overlap; a dense matmul burst
after x0 flips the PE clock gate early; sum(y^2) is taken per half-group
(last four on the then-idle DVE); stats cross 8 cores via a warmed-up
split AllReduce; the tail applies BN in eight [C,2048] 4x-mode DVE
passes each immediately followed by its 512KB bf16 write on alternating
queues.
"""

import numpy as np
import ml_dtypes

B, C, Himg, Wimg = 8, 128, 128, 128
L = Himg * Wimg            # 16384
NCORES = 8
NCH = 512                  # psum chunk columns
NCHUNK = L // NCH          # 32
NW = 2048                  # elementwise window columns (4 chunks)
NGRP = L // NW             # 8
EPS_BN = 1e-5
EPS_NORM = 1e-12
NTOT = float(B * L)        # batchnorm population per channel

_CACHE = {}


def _make_patched_tc():
    """TileContext whose exit drain splits sem waits one-per-Drain.

    The walrus build in this container rejects Drain instructions carrying
    more than one sem wait ("Too many sync wait commands"). Stock
    TileContext attaches the whole global vector clock to a single tail
    Drain; emit one Drain per outstanding proc instead.
    """
    import bass_rust
    import concourse.tile as tile
    from concourse.vector_clock import ScopedClock

    class PatchedTC(tile.TileContext):
        def _drain_and_barrier(self, tick_clock, wait_clock):
            gc = list(tick_clock.global_clock)
            for i, v in enumerate(gc):
                if v:
                    single = [0] * len(gc)
                    single[i] = v
                    d = self.nc.sync.drain()
                    wait_clock.add_sem_waits(
                        d.ins, ScopedClock({None: bass_rust.VectorClock(single)})
                    )
            self.nc.all_engine_barrier()
            assert self.sems is not None
            popped = self.nc._tile_sem_poison_stack.pop()
            assert popped is self._sem_poison
            self.nc.clear_and_free_semaphores(list(self.sems.allocated().values()))
            self.nc.all_engine_barrier()

    return PatchedTC


def _split_excess_waits(nc):
    """Walrus here allows one sem wait per instruction; hoist extras onto
    same-engine NoOps inserted immediately before the instruction."""
    import bass_rust

    nid = 0
    for blk in nc.main_func.blocks:
        out = []
        for ins in blk.instructions:
            si = getattr(ins, "sync_info", None)
            waits = list(si.on_wait) if si is not None else []
            if len(waits) > 1:
                for w in waits[:-1]:
                    nid += 1
                    nop = bass_rust.InstNoOp(
                        name=f"I-waitsplit-{nid}", ins=[], outs=[])
                    nop.engine = ins.engine
                    nop.sync_info = bass_rust.SyncInfo(
                        on_wait=[w], on_update=[])
                    nc.register_instruction(nop, overwrite=True)
                    out.append(nop)
                si.on_wait = [waits[-1]]
                ins.sync_info = si
            out.append(ins)
        blk.instructions = out


def _build_program():
    import concourse.bass as bass
    import concourse.mybir as mybir

    PatchedTC = _make_patched_tc()

    f32 = mybir.dt.float32
    bf16 = mybir.dt.bfloat16
    Alu = mybir.AluOpType
    Act = mybir.ActivationFunctionType

    nc = bass.Bass(target_bir_lowering=False, num_devices=NCORES)

    x_ext = nc.declare_dram_parameter("x", [C, L], bf16, isOutput=False)
    wb3t_ext = nc.declare_dram_parameter("wb3t", [C, C], bf16, isOutput=False)
    wa2t_ext = nc.declare_dram_parameter("wa2t", [C, C], bf16, isOutput=False)
    wdt_ext = nc.declare_dram_parameter("wdt", [C, C], bf16, isOutput=False)
    c2t4_ext = nc.declare_dram_parameter("c2t4", [C, C], bf16, isOutput=False)
    mqt_ext = nc.declare_dram_parameter("mqt", [C, 32], bf16, isOutput=False)
    mpt_ext = nc.declare_dram_parameter("mpt", [C, 32], bf16, isOutput=False)
    mrt_ext = nc.declare_dram_parameter("mrt", [C, 32], bf16, isOutput=False)
    b1t_ext = nc.declare_dram_parameter("b1t", [C, 1], f32, isOutput=False)
    bout_ext = nc.declare_dram_parameter("bout", [C, 1], f32, isOutput=False)
    gb_ext = nc.declare_dram_parameter("gb", [C, 2], f32, isOutput=False)
    y_ext = nc.declare_dram_parameter("y", [C, L], bf16, isOutput=True)

    with PatchedTC(nc) as tc:
        with (
            tc.tile_pool(name="wp", bufs=1) as wp,
            tc.tile_pool(name="big", bufs=1) as big,
            tc.tile_pool(name="win", bufs=5) as win,
            tc.tile_pool(name="dwin", bufs=2) as dwin,
            tc.tile_pool(name="sm", bufs=1) as sm,
            tc.tile_pool(name="sq", bufs=1) as sqp,
            tc.tile_pool(name="ow", bufs=6) as owp,
            tc.tile_pool(name="yps", bufs=2, space="PSUM") as yps,
            tc.tile_pool(name="hps", bufs=3, space="PSUM") as hps,
            tc.tile_pool(name="sps", bufs=1, space="PSUM") as sps,
            tc.tile_pool(name="dram", bufs=1, space="DRAM") as dram,
        ):
            # ---- weights to SBUF ----
            wb3t = wp.tile([C, C], bf16)
            wa2t = wp.tile([C, C], bf16)
            wdt = wp.tile([C, C], bf16)
            c2t4 = wp.tile([C, C], bf16)
            # pad keeps the big-pool base at the baseline offset; moving it
            # was measured to slow every DVE op by ~20% (SBUF alignment)
            lpad = wp.tile([C, 371], f32)
            mqt = wp.tile([C, 32], bf16)
            mpt = wp.tile([C, 32], bf16)
            mrt = wp.tile([C, 32], bf16)
            b1t = wp.tile([C, 1], f32)
            bout = wp.tile([C, 1], f32)
            gb = wp.tile([C, 2], f32)
            # small ramp weights ride the scalar queue; the sync queue is
            # reserved for x so x0/x1 land as early as possible (the whole
            # DVE pipeline is gated on them)
            for t, e in [(b1t, b1t_ext), (wb3t, wb3t_ext), (mqt, mqt_ext),
                         (mpt, mpt_ext), (mrt, mrt_ext)]:
                nc.scalar.dma_start(out=t, in_=e[:])

            nc.vector.memset(lpad, 0.0)   # keep the pad tile allocated

            # warmup collective: pays the ~11us ncfw startup while x loads
            wu_sb = sm.tile([1, 2], f32)
            nc.vector.memset(wu_sb, 0.0)
            wu_in = dram.tile([1, 2], f32)
            wu_out = dram.tile([1, 2], f32)
            nc.sync.dma_start(out=wu_in[:], in_=wu_sb)
            nc.gpsimd.collective_compute(
                "AllReduce", Alu.add,
                replica_groups=[list(range(NCORES))],
                ins=[wu_in.opt()], outs=[wu_out.opt()])

            # ---- big SBUF arrays ----
            xbf = big.tile([C, L], bf16)     # x (bf16, cast on host)
            ypre = big.tile([C, L], bf16)    # pre-BN output (bias included)
            h1sb = big.tile([C, NGRP * NCH], bf16)  # silu(h1) packed 4ch/grp

            NACC = 16
            ysum2 = sm.tile([C, NACC], f32)  # per-half-group sum of y
            ysq = sm.tile([C, NACC], f32)    # per-half-group sum of y^2

            sqdump = sqp.tile([C, NW], bf16, tag="sqd")
            dvsq = sqp.tile([C, 2 * NCH], bf16, tag="dvsq")

            # ---- ACT table prefetch (all sets used mid-run) ----
            scr1 = sm.tile([C, 1], f32)
            for fn in (Act.Copy, Act.Identity, Act.Square, Act.Silu):
                nc.scalar.activation(scr1, b1t, fn)

            # ---- x loads: x0 in two halves then x1, first on sync (they
            # gate windows(0); the first 256KB piece lands ~3us sooner
            # under the 8-core HBM load contention); odd windows 3/5/7 on
            # the scalar queue behind its small weights, the rest on sync
            HW2 = NW // 2
            nc.sync.dma_start(out=xbf[:, 0:HW2], in_=x_ext[:, 0:HW2])
            nc.sync.dma_start(out=xbf[:, HW2:NW], in_=x_ext[:, HW2:NW])
            for g, eng in [(1, nc.sync), (2, nc.sync),
                           (3, nc.scalar), (4, nc.sync), (5, nc.scalar),
                           (6, nc.sync), (7, nc.scalar)]:
                lo, hi = g * NW, (g + 1) * NW
                eng.dma_start(out=xbf[:, lo:hi], in_=x_ext[:, lo:hi])
            for t, e in [(wa2t, wa2t_ext), (wdt, wdt_ext), (c2t4, c2t4_ext),
                         (bout, bout_ext), (gb, gb_ext)]:
                nc.sync.dma_start(out=t, in_=e[:])
            # HAM warmup: a dense ~3.4us burst right after x0 flips the PE
            # clock gate to 2.4GHz (scattered tiny matmuls never sustain a
            # full SHORT window), then keep-alives paced by the x arrivals
            # stop the MID window from re-throttling before the ramp.
            ham_ps = sps.tile([C, NCH], f32, tag="sp")
            for i in range(8):
                nc.tensor.matmul(ham_ps, wb3t, xbf[:, 0:NCH],
                                 start=True, stop=True)
            for g in range(1, 7):
                nc.tensor.matmul(ham_ps[:, 0:128], wb3t,
                                 xbf[:, g * NW:g * NW + 128],
                                 start=True, stop=True)

            # ---- main pipeline over groups ----
            def windows_g0():
                """Group-0 windows split into an x0-only body plus x1
                tails, so DVE starts the moment x0 lands instead of
                waiting for x1 (every full-window op straddles 1-128
                columns into window 1)."""
                sh = win.tile([C, NW], bf16, tag="sh")
                sv = win.tile([C, NW], bf16, tag="sv")
                dh = dwin.tile([C, NW + 4], bf16, tag="dh")
                dv = dwin.tile([C, NW + 128], bf16, tag="dv")
                Hw = win.tile([C, NW], bf16, tag="Hw")
                Vw = win.tile([C, NW], bf16, tag="Vw")
                dhu = dh.bitcast(mybir.dt.uint16)
                dvu = dv.bitcast(mybir.dt.uint16)
                T = NW - 1   # last col readable from x0 is NW-1
                M = NW // 2  # x0 arrives in two halves; cols < M land first
                h3 = Hw.rearrange("p (r c) -> p r c", c=Wimg)
                d3 = dh[:, 0:NW].rearrange("p (r c) -> p r c", c=Wimg)
                RH = M // Wimg   # image rows covered by the first half

                # ---- early body (reads xbf cols < M only) ----
                nc.vector.tensor_tensor(sh[:, 1:M - 1], xbf[:, 0:M - 2],
                                        xbf[:, 2:M], Alu.add)
                nc.vector.tensor_copy(sh[:, 0:1], xbf[:, 1:2])
                nc.vector.tensor_tensor(sv[:, 128:M - 128],
                                        xbf[:, 0:M - 256],
                                        xbf[:, 256:M], Alu.add)
                nc.vector.tensor_copy(sv[:, 0:128], xbf[:, 128:256])
                nc.vector.tensor_tensor(dh[:, 1:M], xbf[:, 1:M],
                                        xbf[:, 0:M - 1], Alu.subtract)
                nc.vector.memset(dh[:, 0:1], 0.0)
                nc.vector.tensor_scalar(dhu[:, 0:M], dhu[:, 0:M], 0x7FFF,
                                        None, Alu.bitwise_and)
                nc.vector.tensor_tensor(Hw[:, 0:M - 1], dh[:, 0:M - 1],
                                        dh[:, 1:M], Alu.add)
                nc.scalar.activation(h3[:, 0:RH, 0:1], d3[:, 0:RH, 1:2],
                                     Act.Copy, scale=2.0)
                nc.scalar.activation(h3[:, 0:RH, Wimg - 1:Wimg],
                                     d3[:, 0:RH, Wimg - 1:Wimg],
                                     Act.Copy, scale=2.0)
                nc.vector.tensor_tensor(dv[:, 128:M], xbf[:, 128:M],
                                        xbf[:, 0:M - 128], Alu.subtract)
                nc.vector.tensor_scalar(dvu[:, 128:M], dvu[:, 128:M],
                                        0x7FFF, None, Alu.bitwise_and)
                nc.vector.tensor_tensor(Vw[:, 128:M - 128], dv[:, 128:M - 128],
                                        dv[:, 256:M], Alu.add)
                nc.vector.tensor_scalar(Vw[:, 0:128], dv[:, 128:256], 2.0,
                                        None, Alu.mult)

                # ---- late body (reads xbf cols [M, NW) — second x0 half)
                nc.vector.tensor_tensor(sh[:, M - 1:T], xbf[:, M - 2:T - 1],
                                        xbf[:, M:T + 1], Alu.add)
                nc.vector.tensor_tensor(sv[:, M - 128:NW - 128],
                                        xbf[:, M - 256:NW - 256],
                                        xbf[:, M:NW], Alu.add)
                nc.vector.tensor_tensor(dh[:, M:NW], xbf[:, M:NW],
                                        xbf[:, M - 1:NW - 1], Alu.subtract)
                nc.vector.tensor_scalar(dhu[:, M:NW], dhu[:, M:NW], 0x7FFF,
                                        None, Alu.bitwise_and)
                # H late [M:NW-1); cols M-1 and NW-1 (rows 7/15, col 127)
                # take their final value from the row-edge fixups
                nc.vector.tensor_tensor(Hw[:, M:NW - 1], dh[:, M:NW - 1],
                                        dh[:, M + 1:NW], Alu.add)
                nc.scalar.activation(h3[:, RH:, 0:1], d3[:, RH:, 1:2],
                                     Act.Copy, scale=2.0)
                nc.scalar.activation(h3[:, RH:, Wimg - 1:Wimg],
                                     d3[:, RH:, Wimg - 1:Wimg],
                                     Act.Copy, scale=2.0)
                nc.vector.tensor_tensor(dv[:, M:NW], xbf[:, M:NW],
                                        xbf[:, M - 128:NW - 128], Alu.subtract)
                nc.vector.tensor_scalar(dvu[:, M:NW], dvu[:, M:NW],
                                        0x7FFF, None, Alu.bitwise_and)
                nc.vector.tensor_tensor(Vw[:, M - 128:NW - 128],
                                        dv[:, M - 128:NW - 128],
                                        dv[:, M:NW], Alu.add)

                # ---- tails (read xbf cols >= NW, i.e. window 1) ----
                nc.vector.tensor_tensor(sh[:, T:NW], xbf[:, T - 1:NW - 1],
                                        xbf[:, T + 1:NW + 1], Alu.add)
                nc.vector.tensor_tensor(sv[:, NW - 128:NW],
                                        xbf[:, NW - 256:NW - 128],
                                        xbf[:, NW:NW + 128], Alu.add)
                nc.vector.tensor_tensor(dh[:, NW:NW + 1], xbf[:, NW:NW + 1],
                                        xbf[:, NW - 1:NW], Alu.subtract)
                nc.vector.tensor_scalar(dhu[:, NW:NW + 1], dhu[:, NW:NW + 1],
                                        0x7FFF, None, Alu.bitwise_and)
                nc.vector.tensor_tensor(dv[:, NW:NW + 128], xbf[:, NW:NW + 128],
                                        xbf[:, NW - 128:NW], Alu.subtract)
                nc.vector.tensor_scalar(dvu[:, NW:NW + 128], dvu[:, NW:NW + 128],
                                        0x7FFF, None, Alu.bitwise_and)
                nc.vector.tensor_tensor(Vw[:, NW - 128:NW], dv[:, NW - 128:NW],
                                        dv[:, NW:NW + 128], Alu.add)
                return sh, sv, Hw, Vw

            def windows_g7():
                """Group-7 windows split at column 1024 so the PE can run
                main_half(7,0) (which reads only cols [0:1024)) while DVE
                still computes the second half — shortening the
                end-of-main chain that gates the stats AllReduce."""
                G0 = (NGRP - 1) * NW
                sh = win.tile([C, NW], bf16, tag="sh")
                sv = win.tile([C, NW], bf16, tag="sv")
                dh = dwin.tile([C, NW + 4], bf16, tag="dh")
                dv = dwin.tile([C, NW + 128], bf16, tag="dv")
                Hw = win.tile([C, NW], bf16, tag="Hw")
                Vw = win.tile([C, NW], bf16, tag="Vw")
                dhu = dh.bitcast(mybir.dt.uint16)
                dvu = dv.bitcast(mybir.dt.uint16)
                M = NW // 2
                h3 = Hw.rearrange("p (r c) -> p r c", c=Wimg)
                d3 = dh[:, 0:NW].rearrange("p (r c) -> p r c", c=Wimg)
                RH = M // Wimg

                # ---- first half: everything main_half(7,0) reads ----
                nc.vector.tensor_tensor(sh[:, 0:M], xbf[:, G0 - 1:G0 + M - 1],
                                        xbf[:, G0 + 1:G0 + M + 1], Alu.add)
                nc.vector.tensor_tensor(sv[:, 0:M],
                                        xbf[:, G0 - 128:G0 + M - 128],
                                        xbf[:, G0 + 128:G0 + M + 128],
                                        Alu.add)
                nc.vector.tensor_tensor(dh[:, 0:M + 1], xbf[:, G0:G0 + M + 1],
                                        xbf[:, G0 - 1:G0 + M], Alu.subtract)
                nc.vector.tensor_scalar(dhu[:, 0:M + 1], dhu[:, 0:M + 1],
                                        0x7FFF, None, Alu.bitwise_and)
                nc.vector.tensor_tensor(Hw[:, 0:M], dh[:, 0:M],
                                        dh[:, 1:M + 1], Alu.add)
                nc.scalar.activation(h3[:, 0:RH, 0:1], d3[:, 0:RH, 1:2],
                                     Act.Copy, scale=2.0)
                nc.scalar.activation(h3[:, 0:RH, Wimg - 1:Wimg],
                                     d3[:, 0:RH, Wimg - 1:Wimg],
                                     Act.Copy, scale=2.0)
                nc.vector.tensor_tensor(dv[:, 0:M + 128],
                                        xbf[:, G0:G0 + M + 128],
                                        xbf[:, G0 - 128:G0 + M], Alu.subtract)
                nc.vector.tensor_scalar(dvu[:, 0:M + 128], dvu[:, 0:M + 128],
                                        0x7FFF, None, Alu.bitwise_and)
                nc.vector.tensor_tensor(Vw[:, 0:M], dv[:, 0:M],
                                        dv[:, 128:M + 128], Alu.add)

                # ---- second half ----
                nc.vector.tensor_tensor(sh[:, M:NW - 1],
                                        xbf[:, G0 + M - 1:L - 2],
                                        xbf[:, G0 + M + 1:L], Alu.add)
                nc.vector.tensor_copy(sh[:, NW - 1:NW], xbf[:, L - 2:L - 1])
                nc.vector.tensor_tensor(sv[:, M:NW - 128],
                                        xbf[:, G0 + M - 128:L - 256],
                                        xbf[:, G0 + M + 128:L], Alu.add)
                nc.vector.tensor_copy(sv[:, NW - 128:NW],
                                      xbf[:, L - 256:L - 128])
                nc.vector.tensor_tensor(dh[:, M + 1:NW], xbf[:, G0 + M + 1:L],
                                        xbf[:, G0 + M:L - 1], Alu.subtract)
                nc.vector.tensor_scalar(dhu[:, M + 1:NW], dhu[:, M + 1:NW],
                                        0x7FFF, None, Alu.bitwise_and)
                # H[M:NW-1); H[NW-1] (row 15 col 127) comes from the fixup
                nc.vector.tensor_tensor(Hw[:, M:NW - 1], dh[:, M:NW - 1],
                                        dh[:, M + 1:NW], Alu.add)
                nc.scalar.activation(h3[:, RH:, 0:1], d3[:, RH:, 1:2],
                                     Act.Copy, scale=2.0)
                nc.scalar.activation(h3[:, RH:, Wimg - 1:Wimg],
                                     d3[:, RH:, Wimg - 1:Wimg],
                                     Act.Copy, scale=2.0)
                nc.vector.tensor_tensor(dv[:, M + 128:NW],
                                        xbf[:, G0 + M + 128:L],
                                        xbf[:, G0 + M:L - 128], Alu.subtract)
                nc.vector.tensor_scalar(dvu[:, M + 128:NW], dvu[:, M + 128:NW],
                                        0x7FFF, None, Alu.bitwise_and)
                nc.vector.tensor_tensor(Vw[:, M:NW - 128], dv[:, M:NW - 128],
                                        dv[:, M + 128:NW], Alu.add)
                nc.vector.tensor_scalar(Vw[:, NW - 128:NW],
                                        dv[:, NW - 128:NW], 2.0, None,
                                        Alu.mult)
                return sh, sv, Hw, Vw

            def windows_rest(g):
                """sh, sv, dh/|dh|/H, dv/|dv|/V for group g (DVE)."""
                G0 = g * NW
                sh = win.tile([C, NW], bf16, tag="sh")
                # s_h[t] = x[l-1] + x[l+1]
                ha = 1 if g == 0 else 0
                hb = NW - 1 if g == NGRP - 1 else NW
                nc.vector.tensor_tensor(sh[:, ha:hb],
                                        xbf[:, G0 + ha - 1:G0 + hb - 1],
                                        xbf[:, G0 + ha + 1:G0 + hb + 1],
                                        Alu.add)
                if g == 0:
                    nc.vector.tensor_copy(sh[:, 0:1], xbf[:, 1:2])
                if g == NGRP - 1:
                    nc.vector.tensor_copy(sh[:, NW - 1:NW],
                                          xbf[:, L - 2:L - 1])
                sv = win.tile([C, NW], bf16, tag="sv")
                # s_v[t] = x[l-128] + x[l+128]
                va = 128 if g == 0 else 0
                vb = NW - 128 if g == NGRP - 1 else NW
                nc.vector.tensor_tensor(sv[:, va:vb],
                                        xbf[:, G0 + va - 128:G0 + vb - 128],
                                        xbf[:, G0 + va + 128:G0 + vb + 128],
                                        Alu.add)
                if g == 0:
                    nc.vector.tensor_copy(sv[:, 0:128], xbf[:, 128:256])
                if g == NGRP - 1:
                    nc.vector.tensor_copy(sv[:, NW - 128:NW],
                                          xbf[:, L - 256:L - 128])

                dh = dwin.tile([C, NW + 4], bf16, tag="dh")
                dv = dwin.tile([C, NW + 128], bf16, tag="dv")
                Hw = win.tile([C, NW], bf16, tag="Hw")

                # d_h[t] = |x[G0+t] - x[G0+t-1]|, t in [a, e)
                a = 1 if g == 0 else 0
                e = NW if g == NGRP - 1 else NW + 1
                nc.vector.tensor_tensor(dh[:, a:e], xbf[:, G0 + a:G0 + e],
                                        xbf[:, G0 + a - 1:G0 + e - 1],
                                        Alu.subtract)
                if g == 0:
                    nc.vector.memset(dh[:, 0:1], 0.0)
                dhu = dh.bitcast(mybir.dt.uint16)
                nc.vector.tensor_scalar(dhu[:, 0:e], dhu[:, 0:e], 0x7FFF,
                                        None, Alu.bitwise_and)
                # H[t] = d_h[t] + d_h[t+1], edges fixed per image row
                he = NW if g < NGRP - 1 else NW - 1
                nc.vector.tensor_tensor(Hw[:, 0:he], dh[:, 0:he],
                                        dh[:, 1:he + 1], Alu.add)
                h3 = Hw.rearrange("p (r c) -> p r c", c=Wimg)
                d3 = dh[:, 0:NW].rearrange("p (r c) -> p r c", c=Wimg)
                # row-edge fixups ride the slack ACT engine, not DVE
                nc.scalar.activation(h3[:, :, 0:1], d3[:, :, 1:2],
                                     Act.Copy, scale=2.0)
                nc.scalar.activation(h3[:, :, Wimg - 1:Wimg],
                                     d3[:, :, Wimg - 1:Wimg],
                                     Act.Copy, scale=2.0)

                # d_v[t] = |x[G0+t] - x[G0+t-128]|, t in [av, ev)
                av = 128 if g == 0 else 0
                ev = NW if g == NGRP - 1 else NW + 128
                nc.vector.tensor_tensor(dv[:, av:ev], xbf[:, G0 + av:G0 + ev],
                                        xbf[:, G0 + av - 128:G0 + ev - 128],
                                        Alu.subtract)
                dvu = dv.bitcast(mybir.dt.uint16)
                nc.vector.tensor_scalar(dvu[:, av:ev], dvu[:, av:ev], 0x7FFF,
                                        None, Alu.bitwise_and)
                # V[t] = d_v[t] + d_v[t+128], first/last image row fixed
                Vw = win.tile([C, NW], bf16, tag="Vw")
                vlo = 128 if g == 0 else 0
                vhi = NW - 128 if g == NGRP - 1 else NW
                nc.vector.tensor_tensor(Vw[:, vlo:vhi], dv[:, vlo:vhi],
                                        dv[:, vlo + 128:vhi + 128], Alu.add)
                if g == 0:
                    nc.vector.tensor_scalar(Vw[:, 0:128], dv[:, 128:256], 2.0,
                                            None, Alu.mult)
                if g == NGRP - 1:
                    nc.vector.tensor_scalar(Vw[:, NW - 128:NW],
                                            dv[:, NW - 128:NW], 2.0, None,
                                            Alu.mult)
                return sh, sv, Hw, Vw

            def channel_mms(k):
                """h1 psum for group k: 3 shifted matmuls x 4 col-bands."""
                h1ps = hps.tile([C, NCH], f32)
                for wgt, shift in [(mqt, 0), (mpt, -1), (mrt, +1)]:
                    for j in range(4):
                        n = 4 * k + j
                        n0 = n * NCH
                        lo = n0 + shift
                        hi = n0 + NCH + shift
                        plo, phi = 0, NCH
                        if lo < 0:
                            plo, lo = 1, 0
                        if hi > L:
                            phi, hi = NCH - 1, L
                        nc.tensor.matmul(
                            h1ps[32 * j:32 * j + 32, plo:phi],
                            wgt[:, 0:32], xbf[:, lo:hi],
                            start=(shift == 0), stop=(shift == 1),
                            tile_position=(0, 32 * j))
                return h1ps

            def main_half(k, h, sh, sv, Hw, Vw):
                """Half-group h (2 chunks) of group k: weight-outer matmuls
                into one [C, 1024] psum tile, then one ACT evacuation."""
                ns = [4 * k + 2 * h, 4 * k + 2 * h + 1]
                ps = yps.tile([C, 2 * NCH], f32)
                off = [0, NCH]              # chunk base col inside ps
                woff = 2 * h * NCH          # chunk base col inside window

                for i, n in enumerate(ns):
                    nc.tensor.matmul(ps[:, off[i]:off[i] + NCH], wb3t,
                                     xbf[:, n * NCH:(n + 1) * NCH],
                                     start=True, stop=False)
                for i, n in enumerate(ns):
                    o = woff + i * NCH
                    nc.tensor.matmul(ps[:, off[i]:off[i] + NCH], wa2t,
                                     sh[:, o:o + NCH],
                                     start=False, stop=False)
                for i, n in enumerate(ns):
                    o = woff + i * NCH
                    nc.tensor.matmul(ps[:, off[i]:off[i] + NCH], wa2t,
                                     sv[:, o:o + NCH],
                                     start=False, stop=False)
                    if n == 0:
                        # col-scan wrap: l=j gets x[(h-1)w + j - 1]
                        nc.tensor.matmul(ps[:, off[i] + 1:off[i] + 128], wa2t,
                                         xbf[:, L - Wimg:L - 1],
                                         start=False, stop=False)
                    if n == NCHUNK - 1:
                        # col-scan wrap: l=(h-1)w+j gets x[j+1]
                        nc.tensor.matmul(
                            ps[:, off[i] + NCH - 128:off[i] + NCH - 1], wa2t,
                            xbf[:, 1:128], start=False, stop=False)
                for i, n in enumerate(ns):
                    o = woff + i * NCH
                    nc.tensor.matmul(ps[:, off[i]:off[i] + NCH], wdt,
                                     Hw[:, o:o + NCH], start=False, stop=False)
                for i, n in enumerate(ns):
                    o = woff + i * NCH
                    nc.tensor.matmul(ps[:, off[i]:off[i] + NCH], wdt,
                                     Vw[:, o:o + NCH], start=False, stop=False)
                for i, n in enumerate(ns):
                    j = n % 4
                    nc.tensor.matmul(
                        ps[:, off[i]:off[i] + NCH],
                        c2t4[32 * j:32 * j + 32, :],
                        h1sb[32 * j:32 * j + 32, k * NCH:(k + 1) * NCH],
                        start=False, stop=True, tile_position=(32 * j, 0))

                hidx = 2 * k + h
                n0 = ns[0] * NCH
                nc.scalar.activation(ypre[:, n0:n0 + 2 * NCH], ps,
                                     Act.Identity, bias=bout[:, 0:1],
                                     accum_out=ysum2[:, hidx:hidx + 1])
                if hidx >= 12:
                    # DVE is idle once its windows are done; give it the
                    # last groups' sum-of-squares to unclog the ACT tail
                    nc.vector.scalar_tensor_tensor(
                        dvsq, ypre[:, n0:n0 + 2 * NCH], 1.0,
                        ypre[:, n0:n0 + 2 * NCH], Alu.bypass, Alu.mult,
                        accum_out=ysq[:, hidx:hidx + 1])
                else:
                    nc.scalar.activation(sqdump[:, 0:2 * NCH],
                                         ypre[:, n0:n0 + 2 * NCH], Act.Square,
                                         accum_out=ysq[:, hidx:hidx + 1])

            def do_main(kk):
                sh, sv, Hw, Vw = wins.pop(kk)
                main_half(kk, 0, sh, sv, Hw, Vw)
                main_half(kk, 1, sh, sv, Hw, Vw)

            wins = {}
            h1s = {}
            wins[0] = windows_g0()
            # note: windows_g7 (half-split of the last group) measured
            # ~+2us on the trigger in a clean-clock sample — the extra op
            # overheads outweigh the PE overlap; keep the plain path
            for k in range(NGRP):
                if k >= 1:
                    wins[k] = windows_rest(k)
                h1s[k] = channel_mms(k)
                # evacs of group k-2 are issued BEFORE silu_k: the strict
                # ACT FIFO must not queue them behind a silu that itself
                # waits on PE, or the next group's PSUM reuse stalls PE
                if k >= 2:
                    do_main(k - 2)
                nc.scalar.activation(h1sb[:, k * NCH:(k + 1) * NCH], h1s[k],
                                     Act.Silu, bias=b1t[:, 0:1])
            do_main(NGRP - 2)
            do_main(NGRP - 1)

            # Sqrt table prefetch: loads while the collective runs
            nc.scalar.activation(scr1, b1t, Act.Sqrt)

            # ---- global BN stats: ship the raw [C,32] accumulators the
            # moment their last writers land (DMA waits on sems only, no
            # pre-reduce on the busy engines), AllReduce, reduce after.
            cc_in = dram.tile([C, 2 * NACC], f32)
            cc_out = dram.tile([C, 2 * NACC], f32)
            nc.sync.dma_start(out=cc_in[:, 0:NACC], in_=ysum2)
            nc.scalar.dma_start(out=cc_in[:, NACC:2 * NACC], in_=ysq)
            nc.gpsimd.collective_compute(
                "AllReduce", Alu.add,
                replica_groups=[list(range(NCORES))],
                ins=[cc_in.opt()], outs=[cc_out.opt()])
            statsr = sm.tile([C, 2 * NACC], f32)
            nc.sync.dma_start(out=statsr, in_=cc_out[:])

            sum2 = sm.tile([C, 1], f32)
            sumq = sm.tile([C, 1], f32)
            nc.vector.tensor_reduce(sumq, statsr[:, NACC:2 * NACC],
                                    mybir.AxisListType.X, Alu.add)
            nc.vector.tensor_reduce(sum2, statsr[:, 0:NACC],
                                    mybir.AxisListType.X, Alu.add)
            e2e = sm.tile([C, 1], f32)       # E[y^2] + eps
            nc.vector.tensor_scalar(e2e, sumq, 1.0 / NTOT, EPS_BN,
                                    Alu.mult, Alu.add)
            mean = sm.tile([C, 1], f32)
            nc.vector.tensor_scalar(mean, sum2, 1.0 / NTOT, None, Alu.mult)
            m2 = sm.tile([C, 1], f32)
            nc.vector.tensor_tensor(m2, mean, mean, Alu.mult)
            varep = sm.tile([C, 1], f32)
            nc.vector.tensor_tensor(varep, e2e, m2, Alu.subtract)
            inv = sm.tile([C, 1], f32)
            nc.vector.reciprocal(inv, varep)
            # rest of the chain stays on ACT (per-partition scale/bias)
            # to avoid cross-engine sem hops on the post-collective path
            rstd = sm.tile([C, 1], f32)
            nc.scalar.activation(rstd, inv, Act.Sqrt)
            s_sc = sm.tile([C, 1], f32)
            nc.scalar.activation(s_sc, rstd, Act.Copy, scale=gb[:, 0:1])
            ms = sm.tile([C, 1], f32)
            nc.scalar.activation(ms, mean, Act.Copy, scale=s_sc[:, 0:1])
            t_sc = sm.tile([C, 1], f32)
            nc.scalar.activation(t_sc, ms, Act.Identity, scale=-1.0,
                                 bias=gb[:, 1:2])

            # ---- apply BN in eight [C,2048] 4x-mode DVE passes, each
            # immediately followed by its 512KB bf16 write (2 queues) ----
            for g in range(NGRP):
                lo, hi = g * NW, (g + 1) * NW
                ow = owp.tile([C, NW], bf16, tag="ow")
                nc.vector.tensor_scalar(ow, ypre[:, lo:hi],
                                        s_sc[:, 0:1], t_sc[:, 0:1],
                                        Alu.mult, Alu.add)
                eng = nc.scalar if g % 2 == 0 else nc.sync
                eng.dma_start(out=y_ext[:, lo:hi], in_=ow)

    _split_excess_waits(nc)
    return nc


def _fold_weights(inputs):
    f = np.float32
    W_in = inputs["w_spatial_in"].astype(np.float64)
    W_out = inputs["w_spatial_out"].astype(np.float64)
    dw_sp = inputs["w_dw_spatial"][:, 0, :].astype(np.float64)
    W_proj = inputs["w_out_proj"].astype(np.float64)
    W_mlp2 = inputs["w_mlp2"].astype(np.float64)
    dwt = float(inputs["diff_weight"])

    a_sym = dw_sp[:, 0] + dw_sp[:, 2]
    w1 = dw_sp[:, 1]
    A2 = 0.25 * W_proj @ (W_out * a_sym[None, :]) @ W_in
    B3 = W_proj @ (W_out * w1[None, :]) @ W_in + W_proj
    W_d = 0.25 * dwt * W_proj
    C2 = W_proj @ W_mlp2                     # [c, 32]
    bias_out = W_proj @ inputs["b_mlp2"].astype(np.float64)

    bf = ml_dtypes.bfloat16
    return {
        "wb3t": np.ascontiguousarray(B3.T.astype(bf)),
        "wa2t": np.ascontiguousarray(A2.T.astype(bf)),
        "wdt": np.ascontiguousarray(W_d.T.astype(bf)),
        "c2t4": np.ascontiguousarray(np.tile(C2.T.astype(bf), (4, 1))),
        "b1t": np.ascontiguousarray(
            np.tile(inputs["b_mlp1"].astype(f), 4)[:, None]),
        "bout": np.ascontiguousarray(bias_out.astype(f)[:, None]),
        "gb": np.ascontiguousarray(
            np.stack([inputs["bn_gamma"], inputs["bn_beta"]], 1).astype(f)),
    }


def _channel_weights(inputs, xb):
    """Host-exact rank-1 channel weights for one batch sample.

    m = g g^T / max(||g||, eps)^2 with g = mean(x) collapses the channel
    branch into h1 = silu(u (x) w + b1), w = P.x(-1) + Q.x + R.x(+1):
      u = W1 g / max(||g||,eps)^2,  [P Q R] = Wci^T (taps * (Wco g))
    Returns MQ^T/MP^T/MR^T [C, 32] bf16, where M* = u (x) P/Q/R.
    """
    f64 = np.float64
    g = xb.reshape(C, L).astype(f64).mean(axis=1)
    n = max(np.sqrt((g * g).sum()), EPS_NORM)
    ghat = g / n
    v = inputs["w_ch_out"].astype(f64).T @ ghat        # [C] = Wco^T ghat
    pqr = inputs["w_ch_dw"][:, 0, :].astype(f64) * v[:, None]  # [C,3] taps*v
    PQR = inputs["w_ch_in"].astype(f64).T @ pqr        # [C,3] P,Q,R columns
    u = inputs["w_mlp1"].astype(f64) @ ghat            # [32]
    bf = ml_dtypes.bfloat16
    out = {}
    for name, j in [("mpt", 0), ("mqt", 1), ("mrt", 2)]:
        M = np.outer(u, PQR[:, j])                     # [32, C]
        out[name] = np.ascontiguousarray(M.T.astype(bf))   # [C, 32]
    return out


def _make_in_maps(inputs):
    wmap = _fold_weights(inputs)
    x = inputs["x"].astype(np.float32)  # [B, C, H, W]
    in_maps = []
    for b in range(NCORES):
        m = dict(wmap)
        m["x"] = np.ascontiguousarray(
            x[b].reshape(C, L).astype(ml_dtypes.bfloat16))
        m.update(_channel_weights(inputs, x[b]))
        in_maps.append(m)
    return in_maps


def kernel(**inputs):
    from concourse.bass_utils import run_bass_kernel_spmd

    inputs = {k: np.asarray(v) for k, v in inputs.items()}
    if "nc" not in _CACHE:
        _CACHE["nc"] = _build_program()
    nc = _CACHE["nc"]

    in_maps = _make_in_maps(inputs)
    res = run_bass_kernel_spmd(nc, in_maps, list(range(NCORES)))
    out = np.stack([np.asarray(res.results[b]["y"])
                    .astype(np.float32).reshape(C, Himg, Wimg)
                    for b in range(NCORES)])
    return out


# revision 56
# speedup vs baseline: 1.0473x; 1.0473x over previous
"""OCS fused kernel for Trainium2, data-parallel over batch across 8 cores.

Algebraic restructuring (verified vs reference to ~1e-6 in fp64):

Spatial branch (4 scan orders, shared weights) collapses to a symmetric
5-point stencil with scan-order wrap rules, and the two 1x1 convs fold
through it:
    W_proj @ y_sp = A2 @ sx + (B3 - W_proj) @ x
    sx = sum of 4 flat shifts of x (+/-1 row-major, +/-w) + col-scan wraps
Channel branch: m = g g^T is rank-1 and g = sum(x) depends only on the
input, so the HOST builds the three [128,32] rank-1 weights
MP/MQ/MR = u (x) P/Q/R exactly; the device runs three shifted matmuls,
a silu, and one [128,32] matmul.
Diff branch: |x - nb| terms are shared between opposite directions, so one
|dx| array per axis + shifted adds gives the 4-neighbor abs-diff sum S;
W_proj folds in as W_d @ S.
BatchNorm: per-core partial (sum, sumsq) in two pieces -> two 1KB
AllReduces (the first mid-run absorbs the cross-core skew so the final
one is short) -> affine apply.

Pipeline layout: x loads ride BOTH hardware DMA queues (even windows on
sync, odd on scalar); per 2048-col group the DVE window ops, the PE
channel matmuls and the PE main matmuls overlap; a dense matmul burst
after x0 flips the PE clock gate early; sum(y^2) is taken per half-group
(last four on the then-idle DVE); stats cross 8 cores via a warmed-up
split AllReduce; the tail applies BN in eight [C,2048] 4x-mode DVE
passes each immediately followed by its 512KB bf16 write on alternating
queues.
"""

import numpy as np
import ml_dtypes

B, C, Himg, Wimg = 8, 128, 128, 128
L = Himg * Wimg            # 16384
NCORES = 8
NCH = 512                  # psum chunk columns
NCHUNK = L // NCH          # 32
NW = 2048                  # elementwise window columns (4 chunks)
NGRP = L // NW             # 8
EPS_BN = 1e-5
EPS_NORM = 1e-12
NTOT = float(B * L)        # batchnorm population per channel

_CACHE = {}


def _make_patched_tc():
    """TileContext whose exit drain splits sem waits one-per-Drain.

    The walrus build in this container rejects Drain instructions carrying
    more than one sem wait ("Too many sync wait commands"). Stock
    TileContext attaches the whole global vector clock to a single tail
    Drain; emit one Drain per outstanding proc instead.
    """
    import bass_rust
    import concourse.tile as tile
    from concourse.vector_clock import ScopedClock

    class PatchedTC(tile.TileContext):
        def _drain_and_barrier(self, tick_clock, wait_clock):
            gc = list(tick_clock.global_clock)
            for i, v in enumerate(gc):
                if v:
                    single = [0] * len(gc)
                    single[i] = v
                    d = self.nc.sync.drain()
                    wait_clock.add_sem_waits(
                        d.ins, ScopedClock({None: bass_rust.VectorClock(single)})
                    )
            self.nc.all_engine_barrier()
            assert self.sems is not None
            popped = self.nc._tile_sem_poison_stack.pop()
            assert popped is self._sem_poison
            self.nc.clear_and_free_semaphores(list(self.sems.allocated().values()))
            self.nc.all_engine_barrier()

    return PatchedTC


def _split_excess_waits(nc):
    """Walrus here allows one sem wait per instruction; hoist extras onto
    same-engine NoOps inserted immediately before the instruction."""
    import bass_rust

    nid = 0
    for blk in nc.main_func.blocks:
        out = []
        for ins in blk.instructions:
            si = getattr(ins, "sync_info", None)
            waits = list(si.on_wait) if si is not None else []
            if len(waits) > 1:
                for w in waits[:-1]:
                    nid += 1
                    nop = bass_rust.InstNoOp(
                        name=f"I-waitsplit-{nid}", ins=[], outs=[])
                    nop.engine = ins.engine
                    nop.sync_info = bass_rust.SyncInfo(
                        on_wait=[w], on_update=[])
                    nc.register_instruction(nop, overwrite=True)
                    out.append(nop)
                si.on_wait = [waits[-1]]
                ins.sync_info = si
            out.append(ins)
        blk.instructions = out


def _build_program():
    import concourse.bass as bass
    import concourse.mybir as mybir

    PatchedTC = _make_patched_tc()

    f32 = mybir.dt.float32
    bf16 = mybir.dt.bfloat16
    Alu = mybir.AluOpType
    Act = mybir.ActivationFunctionType

    nc = bass.Bass(target_bir_lowering=False, num_devices=NCORES)

    x_ext = nc.declare_dram_parameter("x", [C, L], bf16, isOutput=False)
    wb3t_ext = nc.declare_dram_parameter("wb3t", [C, C], bf16, isOutput=False)
    wa2t_ext = nc.declare_dram_parameter("wa2t", [C, C], bf16, isOutput=False)
    wdt_ext = nc.declare_dram_parameter("wdt", [C, C], bf16, isOutput=False)
    c2t4_ext = nc.declare_dram_parameter("c2t4", [C, C], bf16, isOutput=False)
    mqt_ext = nc.declare_dram_parameter("mqt", [C, 32], bf16, isOutput=False)
    mpt_ext = nc.declare_dram_parameter("mpt", [C, 32], bf16, isOutput=False)
    mrt_ext = nc.declare_dram_parameter("mrt", [C, 32], bf16, isOutput=False)
    b1t_ext = nc.declare_dram_parameter("b1t", [C, 1], f32, isOutput=False)
    bout_ext = nc.declare_dram_parameter("bout", [C, 1], f32, isOutput=False)
    gb_ext = nc.declare_dram_parameter("gb", [C, 2], f32, isOutput=False)
    y_ext = nc.declare_dram_parameter("y", [C, L], bf16, isOutput=True)

    with PatchedTC(nc) as tc:
        with (
            tc.tile_pool(name="wp", bufs=1) as wp,
            tc.tile_pool(name="big", bufs=1) as big,
            tc.tile_pool(name="win", bufs=5) as win,
            tc.tile_pool(name="dwin", bufs=2) as dwin,
            tc.tile_pool(name="sm", bufs=1) as sm,
            tc.tile_pool(name="sq", bufs=1) as sqp,
            tc.tile_pool(name="ow", bufs=6) as owp,
            tc.tile_pool(name="st", bufs=1) as stp,
            tc.tile_pool(name="yps", bufs=2, space="PSUM") as yps,
            tc.tile_pool(name="hps", bufs=3, space="PSUM") as hps,
            tc.tile_pool(name="sps", bufs=1, space="PSUM") as sps,
            tc.tile_pool(name="dram", bufs=1, space="DRAM") as dram,
        ):
            # ---- weights to SBUF ----
            wb3t = wp.tile([C, C], bf16)
            wa2t = wp.tile([C, C], bf16)
            wdt = wp.tile([C, C], bf16)
            c2t4 = wp.tile([C, C], bf16)
            # pad keeps the big-pool base at the baseline offset; moving it
            # was measured to slow every DVE op by ~20% (SBUF alignment)
            lpad = wp.tile([C, 371], f32)
            mqt = wp.tile([C, 32], bf16)
            mpt = wp.tile([C, 32], bf16)
            mrt = wp.tile([C, 32], bf16)
            b1t = wp.tile([C, 1], f32)
            bout = wp.tile([C, 1], f32)
            gb = wp.tile([C, 2], f32)
            # small ramp weights ride the scalar queue; the sync queue is
            # reserved for x so x0/x1 land as early as possible (the whole
            # DVE pipeline is gated on them)
            for t, e in [(b1t, b1t_ext), (wb3t, wb3t_ext), (mqt, mqt_ext),
                         (mpt, mpt_ext), (mrt, mrt_ext)]:
                nc.scalar.dma_start(out=t, in_=e[:])

            nc.vector.memset(lpad, 0.0)   # keep the pad tile allocated

            # warmup collective: pays the ~11us ncfw startup while x loads
            wu_sb = sm.tile([1, 2], f32)
            nc.vector.memset(wu_sb, 0.0)
            wu_in = dram.tile([1, 2], f32)
            wu_out = dram.tile([1, 2 * NCORES], f32)
            nc.sync.dma_start(out=wu_in[:], in_=wu_sb)
            nc.gpsimd.collective_compute(
                "AllGather", Alu.bypass,
                replica_groups=[list(range(NCORES))],
                ins=[wu_in.opt()], outs=[wu_out.opt()], cc_dim="Free")

            # ---- big SBUF arrays ----
            xbf = big.tile([C, L], bf16)     # x (bf16, cast on host)
            ypre = big.tile([C, L], bf16)    # pre-BN output (bias included)
            h1sb = big.tile([C, NGRP * NCH], bf16)  # silu(h1) packed 4ch/grp

            NACC = 16
            ysum2 = sm.tile([C, NACC], f32)  # per-half-group sum of y
            ysq = sm.tile([C, NACC], f32)    # per-half-group sum of y^2

            sqdump = sqp.tile([C, NW], bf16, tag="sqd")
            dvsq = sqp.tile([C, 2 * NCH], bf16, tag="dvsq")

            # ---- ACT table prefetch (all sets used mid-run) ----
            scr1 = sm.tile([C, 1], f32)
            for fn in (Act.Copy, Act.Identity, Act.Square, Act.Silu):
                nc.scalar.activation(scr1, b1t, fn)

            # ---- x loads: x0 in two halves then x1, first on sync (they
            # gate windows(0); the first 256KB piece lands ~3us sooner
            # under the 8-core HBM load contention); odd windows 3/5/7 on
            # the scalar queue behind its small weights, the rest on sync
            HW2 = NW // 2
            nc.sync.dma_start(out=xbf[:, 0:HW2], in_=x_ext[:, 0:HW2])
            nc.sync.dma_start(out=xbf[:, HW2:NW], in_=x_ext[:, HW2:NW])
            for g, eng in [(1, nc.sync), (2, nc.sync),
                           (3, nc.scalar), (4, nc.sync), (5, nc.scalar),
                           (6, nc.sync), (7, nc.scalar)]:
                lo, hi = g * NW, (g + 1) * NW
                eng.dma_start(out=xbf[:, lo:hi], in_=x_ext[:, lo:hi])
            for t, e in [(wa2t, wa2t_ext), (wdt, wdt_ext), (c2t4, c2t4_ext),
                         (bout, bout_ext), (gb, gb_ext)]:
                nc.sync.dma_start(out=t, in_=e[:])
            # HAM warmup: a dense ~3.4us burst right after x0 flips the PE
            # clock gate to 2.4GHz (scattered tiny matmuls never sustain a
            # full SHORT window), then keep-alives paced by the x arrivals
            # stop the MID window from re-throttling before the ramp.
            ham_ps = sps.tile([C, NCH], f32, tag="sp")
            for i in range(8):
                nc.tensor.matmul(ham_ps, wb3t, xbf[:, 0:NCH],
                                 start=True, stop=True)
            for g in range(1, 7):
                nc.tensor.matmul(ham_ps[:, 0:128], wb3t,
                                 xbf[:, g * NW:g * NW + 128],
                                 start=True, stop=True)

            # ---- main pipeline over groups ----
            def windows_g0():
                """Group-0 windows split into an x0-only body plus x1
                tails, so DVE starts the moment x0 lands instead of
                waiting for x1 (every full-window op straddles 1-128
                columns into window 1)."""
                sh = win.tile([C, NW], bf16, tag="sh")
                sv = win.tile([C, NW], bf16, tag="sv")
                dh = dwin.tile([C, NW + 4], bf16, tag="dh")
                dv = dwin.tile([C, NW + 128], bf16, tag="dv")
                Hw = win.tile([C, NW], bf16, tag="Hw")
                Vw = win.tile([C, NW], bf16, tag="Vw")
                dhu = dh.bitcast(mybir.dt.uint16)
                dvu = dv.bitcast(mybir.dt.uint16)
                T = NW - 1   # last col readable from x0 is NW-1
                M = NW // 2  # x0 arrives in two halves; cols < M land first
                h3 = Hw.rearrange("p (r c) -> p r c", c=Wimg)
                d3 = dh[:, 0:NW].rearrange("p (r c) -> p r c", c=Wimg)
                RH = M // Wimg   # image rows covered by the first half

                # ---- early body (reads xbf cols < M only) ----
                nc.vector.tensor_tensor(sh[:, 1:M - 1], xbf[:, 0:M - 2],
                                        xbf[:, 2:M], Alu.add)
                nc.vector.tensor_copy(sh[:, 0:1], xbf[:, 1:2])
                nc.vector.tensor_tensor(sv[:, 128:M - 128],
                                        xbf[:, 0:M - 256],
                                        xbf[:, 256:M], Alu.add)
                nc.vector.tensor_copy(sv[:, 0:128], xbf[:, 128:256])
                nc.vector.tensor_tensor(dh[:, 1:M], xbf[:, 1:M],
                                        xbf[:, 0:M - 1], Alu.subtract)
                nc.vector.memset(dh[:, 0:1], 0.0)
                nc.vector.tensor_scalar(dhu[:, 0:M], dhu[:, 0:M], 0x7FFF,
                                        None, Alu.bitwise_and)
                nc.vector.tensor_tensor(Hw[:, 0:M - 1], dh[:, 0:M - 1],
                                        dh[:, 1:M], Alu.add)
                nc.scalar.activation(h3[:, 0:RH, 0:1], d3[:, 0:RH, 1:2],
                                     Act.Copy, scale=2.0)
                nc.scalar.activation(h3[:, 0:RH, Wimg - 1:Wimg],
                                     d3[:, 0:RH, Wimg - 1:Wimg],
                                     Act.Copy, scale=2.0)
                nc.vector.tensor_tensor(dv[:, 128:M], xbf[:, 128:M],
                                        xbf[:, 0:M - 128], Alu.subtract)
                nc.vector.tensor_scalar(dvu[:, 128:M], dvu[:, 128:M],
                                        0x7FFF, None, Alu.bitwise_and)
                nc.vector.tensor_tensor(Vw[:, 128:M - 128], dv[:, 128:M - 128],
                                        dv[:, 256:M], Alu.add)
                nc.vector.tensor_scalar(Vw[:, 0:128], dv[:, 128:256], 2.0,
                                        None, Alu.mult)

                # ---- late body (reads xbf cols [M, NW) — second x0 half)
                nc.vector.tensor_tensor(sh[:, M - 1:T], xbf[:, M - 2:T - 1],
                                        xbf[:, M:T + 1], Alu.add)
                nc.vector.tensor_tensor(sv[:, M - 128:NW - 128],
                                        xbf[:, M - 256:NW - 256],
                                        xbf[:, M:NW], Alu.add)
                nc.vector.tensor_tensor(dh[:, M:NW], xbf[:, M:NW],
                                        xbf[:, M - 1:NW - 1], Alu.subtract)
                nc.vector.tensor_scalar(dhu[:, M:NW], dhu[:, M:NW], 0x7FFF,
                                        None, Alu.bitwise_and)
                # H late [M:NW-1); cols M-1 and NW-1 (rows 7/15, col 127)
                # take their final value from the row-edge fixups
                nc.vector.tensor_tensor(Hw[:, M:NW - 1], dh[:, M:NW - 1],
                                        dh[:, M + 1:NW], Alu.add)
                nc.scalar.activation(h3[:, RH:, 0:1], d3[:, RH:, 1:2],
                                     Act.Copy, scale=2.0)
                nc.scalar.activation(h3[:, RH:, Wimg - 1:Wimg],
                                     d3[:, RH:, Wimg - 1:Wimg],
                                     Act.Copy, scale=2.0)
                nc.vector.tensor_tensor(dv[:, M:NW], xbf[:, M:NW],
                                        xbf[:, M - 128:NW - 128], Alu.subtract)
                nc.vector.tensor_scalar(dvu[:, M:NW], dvu[:, M:NW],
                                        0x7FFF, None, Alu.bitwise_and)
                nc.vector.tensor_tensor(Vw[:, M - 128:NW - 128],
                                        dv[:, M - 128:NW - 128],
                                        dv[:, M:NW], Alu.add)

                # ---- tails (read xbf cols >= NW, i.e. window 1) ----
                nc.vector.tensor_tensor(sh[:, T:NW], xbf[:, T - 1:NW - 1],
                                        xbf[:, T + 1:NW + 1], Alu.add)
                nc.vector.tensor_tensor(sv[:, NW - 128:NW],
                                        xbf[:, NW - 256:NW - 128],
                                        xbf[:, NW:NW + 128], Alu.add)
                nc.vector.tensor_tensor(dh[:, NW:NW + 1], xbf[:, NW:NW + 1],
                                        xbf[:, NW - 1:NW], Alu.subtract)
                nc.vector.tensor_scalar(dhu[:, NW:NW + 1], dhu[:, NW:NW + 1],
                                        0x7FFF, None, Alu.bitwise_and)
                nc.vector.tensor_tensor(dv[:, NW:NW + 128], xbf[:, NW:NW + 128],
                                        xbf[:, NW - 128:NW], Alu.subtract)
                nc.vector.tensor_scalar(dvu[:, NW:NW + 128], dvu[:, NW:NW + 128],
                                        0x7FFF, None, Alu.bitwise_and)
                nc.vector.tensor_tensor(Vw[:, NW - 128:NW], dv[:, NW - 128:NW],
                                        dv[:, NW:NW + 128], Alu.add)
                return sh, sv, Hw, Vw

            def windows_g7():
                """Group-7 windows split at column 1024 so the PE can run
                main_half(7,0) (which reads only cols [0:1024)) while DVE
                still computes the second half — shortening the
                end-of-main chain that gates the stats AllReduce."""
                G0 = (NGRP - 1) * NW
                sh = win.tile([C, NW], bf16, tag="sh")
                sv = win.tile([C, NW], bf16, tag="sv")
                dh = dwin.tile([C, NW + 4], bf16, tag="dh")
                dv = dwin.tile([C, NW + 128], bf16, tag="dv")
                Hw = win.tile([C, NW], bf16, tag="Hw")
                Vw = win.tile([C, NW], bf16, tag="Vw")
                dhu = dh.bitcast(mybir.dt.uint16)
                dvu = dv.bitcast(mybir.dt.uint16)
                M = NW // 2
                h3 = Hw.rearrange("p (r c) -> p r c", c=Wimg)
                d3 = dh[:, 0:NW].rearrange("p (r c) -> p r c", c=Wimg)
                RH = M // Wimg

                # ---- first half: everything main_half(7,0) reads ----
                nc.vector.tensor_tensor(sh[:, 0:M], xbf[:, G0 - 1:G0 + M - 1],
                                        xbf[:, G0 + 1:G0 + M + 1], Alu.add)
                nc.vector.tensor_tensor(sv[:, 0:M],
                                        xbf[:, G0 - 128:G0 + M - 128],
                                        xbf[:, G0 + 128:G0 + M + 128],
                                        Alu.add)
                nc.vector.tensor_tensor(dh[:, 0:M + 1], xbf[:, G0:G0 + M + 1],
                                        xbf[:, G0 - 1:G0 + M], Alu.subtract)
                nc.vector.tensor_scalar(dhu[:, 0:M + 1], dhu[:, 0:M + 1],
                                        0x7FFF, None, Alu.bitwise_and)
                nc.vector.tensor_tensor(Hw[:, 0:M], dh[:, 0:M],
                                        dh[:, 1:M + 1], Alu.add)
                nc.scalar.activation(h3[:, 0:RH, 0:1], d3[:, 0:RH, 1:2],
                                     Act.Copy, scale=2.0)
                nc.scalar.activation(h3[:, 0:RH, Wimg - 1:Wimg],
                                     d3[:, 0:RH, Wimg - 1:Wimg],
                                     Act.Copy, scale=2.0)
                nc.vector.tensor_tensor(dv[:, 0:M + 128],
                                        xbf[:, G0:G0 + M + 128],
                                        xbf[:, G0 - 128:G0 + M], Alu.subtract)
                nc.vector.tensor_scalar(dvu[:, 0:M + 128], dvu[:, 0:M + 128],
                                        0x7FFF, None, Alu.bitwise_and)
                nc.vector.tensor_tensor(Vw[:, 0:M], dv[:, 0:M],
                                        dv[:, 128:M + 128], Alu.add)

                # ---- second half ----
                nc.vector.tensor_tensor(sh[:, M:NW - 1],
                                        xbf[:, G0 + M - 1:L - 2],
                                        xbf[:, G0 + M + 1:L], Alu.add)
                nc.vector.tensor_copy(sh[:, NW - 1:NW], xbf[:, L - 2:L - 1])
                nc.vector.tensor_tensor(sv[:, M:NW - 128],
                                        xbf[:, G0 + M - 128:L - 256],
                                        xbf[:, G0 + M + 128:L], Alu.add)
                nc.vector.tensor_copy(sv[:, NW - 128:NW],
                                      xbf[:, L - 256:L - 128])
                nc.vector.tensor_tensor(dh[:, M + 1:NW], xbf[:, G0 + M + 1:L],
                                        xbf[:, G0 + M:L - 1], Alu.subtract)
                nc.vector.tensor_scalar(dhu[:, M + 1:NW], dhu[:, M + 1:NW],
                                        0x7FFF, None, Alu.bitwise_and)
                # H[M:NW-1); H[NW-1] (row 15 col 127) comes from the fixup
                nc.vector.tensor_tensor(Hw[:, M:NW - 1], dh[:, M:NW - 1],
                                        dh[:, M + 1:NW], Alu.add)
                nc.scalar.activation(h3[:, RH:, 0:1], d3[:, RH:, 1:2],
                                     Act.Copy, scale=2.0)
                nc.scalar.activation(h3[:, RH:, Wimg - 1:Wimg],
                                     d3[:, RH:, Wimg - 1:Wimg],
                                     Act.Copy, scale=2.0)
                nc.vector.tensor_tensor(dv[:, M + 128:NW],
                                        xbf[:, G0 + M + 128:L],
                                        xbf[:, G0 + M:L - 128], Alu.subtract)
                nc.vector.tensor_scalar(dvu[:, M + 128:NW], dvu[:, M + 128:NW],
                                        0x7FFF, None, Alu.bitwise_and)
                nc.vector.tensor_tensor(Vw[:, M:NW - 128], dv[:, M:NW - 128],
                                        dv[:, M + 128:NW], Alu.add)
                nc.vector.tensor_scalar(Vw[:, NW - 128:NW],
                                        dv[:, NW - 128:NW], 2.0, None,
                                        Alu.mult)
                return sh, sv, Hw, Vw

            def windows_rest(g):
                """sh, sv, dh/|dh|/H, dv/|dv|/V for group g (DVE)."""
                G0 = g * NW
                sh = win.tile([C, NW], bf16, tag="sh")
                # s_h[t] = x[l-1] + x[l+1]
                ha = 1 if g == 0 else 0
                hb = NW - 1 if g == NGRP - 1 else NW
                nc.vector.tensor_tensor(sh[:, ha:hb],
                                        xbf[:, G0 + ha - 1:G0 + hb - 1],
                                        xbf[:, G0 + ha + 1:G0 + hb + 1],
                                        Alu.add)
                if g == 0:
                    nc.vector.tensor_copy(sh[:, 0:1], xbf[:, 1:2])
                if g == NGRP - 1:
                    nc.vector.tensor_copy(sh[:, NW - 1:NW],
                                          xbf[:, L - 2:L - 1])
                sv = win.tile([C, NW], bf16, tag="sv")
                # s_v[t] = x[l-128] + x[l+128]
                va = 128 if g == 0 else 0
                vb = NW - 128 if g == NGRP - 1 else NW
                nc.vector.tensor_tensor(sv[:, va:vb],
                                        xbf[:, G0 + va - 128:G0 + vb - 128],
                                        xbf[:, G0 + va + 128:G0 + vb + 128],
                                        Alu.add)
                if g == 0:
                    nc.vector.tensor_copy(sv[:, 0:128], xbf[:, 128:256])
                if g == NGRP - 1:
                    nc.vector.tensor_copy(sv[:, NW - 128:NW],
                                          xbf[:, L - 256:L - 128])

                dh = dwin.tile([C, NW + 4], bf16, tag="dh")
                dv = dwin.tile([C, NW + 128], bf16, tag="dv")
                Hw = win.tile([C, NW], bf16, tag="Hw")

                # d_h[t] = |x[G0+t] - x[G0+t-1]|, t in [a, e)
                a = 1 if g == 0 else 0
                e = NW if g == NGRP - 1 else NW + 1
                nc.vector.tensor_tensor(dh[:, a:e], xbf[:, G0 + a:G0 + e],
                                        xbf[:, G0 + a - 1:G0 + e - 1],
                                        Alu.subtract)
                if g == 0:
                    nc.vector.memset(dh[:, 0:1], 0.0)
                dhu = dh.bitcast(mybir.dt.uint16)
                nc.vector.tensor_scalar(dhu[:, 0:e], dhu[:, 0:e], 0x7FFF,
                                        None, Alu.bitwise_and)
                # H[t] = d_h[t] + d_h[t+1], edges fixed per image row
                he = NW if g < NGRP - 1 else NW - 1
                nc.vector.tensor_tensor(Hw[:, 0:he], dh[:, 0:he],
                                        dh[:, 1:he + 1], Alu.add)
                h3 = Hw.rearrange("p (r c) -> p r c", c=Wimg)
                d3 = dh[:, 0:NW].rearrange("p (r c) -> p r c", c=Wimg)
                # row-edge fixups ride the slack ACT engine, not DVE
                nc.scalar.activation(h3[:, :, 0:1], d3[:, :, 1:2],
                                     Act.Copy, scale=2.0)
                nc.scalar.activation(h3[:, :, Wimg - 1:Wimg],
                                     d3[:, :, Wimg - 1:Wimg],
                                     Act.Copy, scale=2.0)

                # d_v[t] = |x[G0+t] - x[G0+t-128]|, t in [av, ev)
                av = 128 if g == 0 else 0
                ev = NW if g == NGRP - 1 else NW + 128
                nc.vector.tensor_tensor(dv[:, av:ev], xbf[:, G0 + av:G0 + ev],
                                        xbf[:, G0 + av - 128:G0 + ev - 128],
                                        Alu.subtract)
                dvu = dv.bitcast(mybir.dt.uint16)
                nc.vector.tensor_scalar(dvu[:, av:ev], dvu[:, av:ev], 0x7FFF,
                                        None, Alu.bitwise_and)
                # V[t] = d_v[t] + d_v[t+128], first/last image row fixed
                Vw = win.tile([C, NW], bf16, tag="Vw")
                vlo = 128 if g == 0 else 0
                vhi = NW - 128 if g == NGRP - 1 else NW
                nc.vector.tensor_tensor(Vw[:, vlo:vhi], dv[:, vlo:vhi],
                                        dv[:, vlo + 128:vhi + 128], Alu.add)
                if g == 0:
                    nc.vector.tensor_scalar(Vw[:, 0:128], dv[:, 128:256], 2.0,
                                            None, Alu.mult)
                if g == NGRP - 1:
                    nc.vector.tensor_scalar(Vw[:, NW - 128:NW],
                                            dv[:, NW - 128:NW], 2.0, None,
                                            Alu.mult)
                return sh, sv, Hw, Vw

            def channel_mms(k):
                """h1 psum for group k: 3 shifted matmuls x 4 col-bands."""
                h1ps = hps.tile([C, NCH], f32)
                for wgt, shift in [(mqt, 0), (mpt, -1), (mrt, +1)]:
                    for j in range(4):
                        n = 4 * k + j
                        n0 = n * NCH
                        lo = n0 + shift
                        hi = n0 + NCH + shift
                        plo, phi = 0, NCH
                        if lo < 0:
                            plo, lo = 1, 0
                        if hi > L:
                            phi, hi = NCH - 1, L
                        nc.tensor.matmul(
                            h1ps[32 * j:32 * j + 32, plo:phi],
                            wgt[:, 0:32], xbf[:, lo:hi],
                            start=(shift == 0), stop=(shift == 1),
                            tile_position=(0, 32 * j))
                return h1ps

            def main_half(k, h, sh, sv, Hw, Vw):
                """Half-group h (2 chunks) of group k: weight-outer matmuls
                into one [C, 1024] psum tile, then one ACT evacuation."""
                ns = [4 * k + 2 * h, 4 * k + 2 * h + 1]
                ps = yps.tile([C, 2 * NCH], f32)
                off = [0, NCH]              # chunk base col inside ps
                woff = 2 * h * NCH          # chunk base col inside window

                for i, n in enumerate(ns):
                    nc.tensor.matmul(ps[:, off[i]:off[i] + NCH], wb3t,
                                     xbf[:, n * NCH:(n + 1) * NCH],
                                     start=True, stop=False)
                for i, n in enumerate(ns):
                    o = woff + i * NCH
                    nc.tensor.matmul(ps[:, off[i]:off[i] + NCH], wa2t,
                                     sh[:, o:o + NCH],
                                     start=False, stop=False)
                for i, n in enumerate(ns):
                    o = woff + i * NCH
                    nc.tensor.matmul(ps[:, off[i]:off[i] + NCH], wa2t,
                                     sv[:, o:o + NCH],
                                     start=False, stop=False)
                    if n == 0:
                        # col-scan wrap: l=j gets x[(h-1)w + j - 1]
                        nc.tensor.matmul(ps[:, off[i] + 1:off[i] + 128], wa2t,
                                         xbf[:, L - Wimg:L - 1],
                                         start=False, stop=False)
                    if n == NCHUNK - 1:
                        # col-scan wrap: l=(h-1)w+j gets x[j+1]
                        nc.tensor.matmul(
                            ps[:, off[i] + NCH - 128:off[i] + NCH - 1], wa2t,
                            xbf[:, 1:128], start=False, stop=False)
                for i, n in enumerate(ns):
                    o = woff + i * NCH
                    nc.tensor.matmul(ps[:, off[i]:off[i] + NCH], wdt,
                                     Hw[:, o:o + NCH], start=False, stop=False)
                for i, n in enumerate(ns):
                    o = woff + i * NCH
                    nc.tensor.matmul(ps[:, off[i]:off[i] + NCH], wdt,
                                     Vw[:, o:o + NCH], start=False, stop=False)
                for i, n in enumerate(ns):
                    j = n % 4
                    nc.tensor.matmul(
                        ps[:, off[i]:off[i] + NCH],
                        c2t4[32 * j:32 * j + 32, :],
                        h1sb[32 * j:32 * j + 32, k * NCH:(k + 1) * NCH],
                        start=False, stop=True, tile_position=(32 * j, 0))

                hidx = 2 * k + h
                n0 = ns[0] * NCH
                nc.scalar.activation(ypre[:, n0:n0 + 2 * NCH], ps,
                                     Act.Identity, bias=bout[:, 0:1],
                                     accum_out=ysum2[:, hidx:hidx + 1])
                if hidx >= 12:
                    # DVE is idle once its windows are done; give it the
                    # last groups' sum-of-squares to unclog the ACT tail
                    nc.vector.scalar_tensor_tensor(
                        dvsq, ypre[:, n0:n0 + 2 * NCH], 1.0,
                        ypre[:, n0:n0 + 2 * NCH], Alu.bypass, Alu.mult,
                        accum_out=ysq[:, hidx:hidx + 1])
                else:
                    nc.scalar.activation(sqdump[:, 0:2 * NCH],
                                         ypre[:, n0:n0 + 2 * NCH], Act.Square,
                                         accum_out=ysq[:, hidx:hidx + 1])

            def do_main(kk):
                sh, sv, Hw, Vw = wins.pop(kk)
                main_half(kk, 0, sh, sv, Hw, Vw)
                main_half(kk, 1, sh, sv, Hw, Vw)

            wins = {}
            h1s = {}
            wins[0] = windows_g0()
            # note: windows_g7 (half-split of the last group) measured
            # ~+2us on the trigger in a clean-clock sample — the extra op
            # overheads outweigh the PE overlap; keep the plain path
            for k in range(NGRP):
                if k >= 1:
                    wins[k] = windows_rest(k)
                h1s[k] = channel_mms(k)
                # evacs of group k-2 are issued BEFORE silu_k: the strict
                # ACT FIFO must not queue them behind a silu that itself
                # waits on PE, or the next group's PSUM reuse stalls PE
                if k >= 2:
                    do_main(k - 2)
                nc.scalar.activation(h1sb[:, k * NCH:(k + 1) * NCH], h1s[k],
                                     Act.Silu, bias=b1t[:, 0:1])
            do_main(NGRP - 2)
            do_main(NGRP - 1)

            # Sqrt table prefetch: loads while the collective runs
            nc.scalar.activation(scr1, b1t, Act.Sqrt)

            # ---- global BN stats: ship the raw [C,32] accumulators the
            # moment their last writers land (DMA waits on sems only, no
            # pre-reduce on the busy engines), AllReduce, reduce after.
            cc_in = dram.tile([C, 2 * NACC], f32)
            # AllGather stacks the 8 contributions along the partition dim:
            # cc_out is [8*C, 32] with core k's block at rows [k*C, (k+1)*C)
            cc_out = dram.tile([NCORES * C, 2 * NACC], f32)
            nc.sync.dma_start(out=cc_in[:, 0:NACC], in_=ysum2)
            nc.scalar.dma_start(out=cc_in[:, NACC:2 * NACC], in_=ysq)
            # AllGather + local tree-sum: fewer mesh stages than AllReduce
            nc.gpsimd.collective_compute(
                "AllGather", Alu.bypass,
                replica_groups=[list(range(NCORES))],
                ins=[cc_in.opt()], outs=[cc_out.opt()])
            sm_pad = sm.tile([C, 2 * NACC], f32)  # keeps sq/ow pool bases
            nc.vector.memset(sm_pad, 0.0)
            W = 2 * NACC  # 32
            statsg = stp.tile([C, 2 * NACC * NCORES], f32)
            for k in range(NCORES):
                eng = nc.sync if k % 2 == 0 else nc.scalar
                eng.dma_start(out=statsg[:, k * W:(k + 1) * W],
                              in_=cc_out[k * C:(k + 1) * C, :])
            t1 = stp.tile([C, 4 * W], f32)
            t2 = stp.tile([C, 2 * W], f32)
            statsr = stp.tile([C, W], f32)
            nc.vector.tensor_tensor(t1, statsg[:, 0:4 * W],
                                    statsg[:, 4 * W:8 * W], Alu.add)
            nc.vector.tensor_tensor(t2, t1[:, 0:2 * W], t1[:, 2 * W:4 * W],
                                    Alu.add)
            nc.vector.tensor_tensor(statsr, t2[:, 0:W], t2[:, W:2 * W],
                                    Alu.add)

            sum2 = sm.tile([C, 1], f32)
            sumq = sm.tile([C, 1], f32)
            nc.vector.tensor_reduce(sumq, statsr[:, NACC:2 * NACC],
                                    mybir.AxisListType.X, Alu.add)
            nc.vector.tensor_reduce(sum2, statsr[:, 0:NACC],
                                    mybir.AxisListType.X, Alu.add)
            e2e = sm.tile([C, 1], f32)       # E[y^2] + eps
            nc.vector.tensor_scalar(e2e, sumq, 1.0 / NTOT, EPS_BN,
                                    Alu.mult, Alu.add)
            mean = sm.tile([C, 1], f32)
            nc.vector.tensor_scalar(mean, sum2, 1.0 / NTOT, None, Alu.mult)
            m2 = sm.tile([C, 1], f32)
            nc.vector.tensor_tensor(m2, mean, mean, Alu.mult)
            varep = sm.tile([C, 1], f32)
            nc.vector.tensor_tensor(varep, e2e, m2, Alu.subtract)
            inv = sm.tile([C, 1], f32)
            nc.vector.reciprocal(inv, varep)
            # rest of the chain stays on ACT (per-partition scale/bias)
            # to avoid cross-engine sem hops on the post-collective path
            rstd = sm.tile([C, 1], f32)
            nc.scalar.activation(rstd, inv, Act.Sqrt)
            s_sc = sm.tile([C, 1], f32)
            nc.scalar.activation(s_sc, rstd, Act.Copy, scale=gb[:, 0:1])
            ms = sm.tile([C, 1], f32)
            nc.scalar.activation(ms, mean, Act.Copy, scale=s_sc[:, 0:1])
            t_sc = sm.tile([C, 1], f32)
            nc.scalar.activation(t_sc, ms, Act.Identity, scale=-1.0,
                                 bias=gb[:, 1:2])

            # ---- apply BN in eight [C,2048] 4x-mode DVE passes, each
            # immediately followed by its 512KB bf16 write (2 queues) ----
            for g in range(NGRP):
                lo, hi = g * NW, (g + 1) * NW
                ow = owp.tile([C, NW], bf16, tag="ow")
                nc.vector.tensor_scalar(ow, ypre[:, lo:hi],
                                        s_sc[:, 0:1], t_sc[:, 0:1],
                                        Alu.mult, Alu.add)
                eng = nc.scalar if g % 2 == 0 else nc.sync
                eng.dma_start(out=y_ext[:, lo:hi], in_=ow)

    _split_excess_waits(nc)
    return nc


def _fold_weights(inputs):
    f = np.float32
    W_in = inputs["w_spatial_in"].astype(np.float64)
    W_out = inputs["w_spatial_out"].astype(np.float64)
    dw_sp = inputs["w_dw_spatial"][:, 0, :].astype(np.float64)
    W_proj = inputs["w_out_proj"].astype(np.float64)
    W_mlp2 = inputs["w_mlp2"].astype(np.float64)
    dwt = float(inputs["diff_weight"])

    a_sym = dw_sp[:, 0] + dw_sp[:, 2]
    w1 = dw_sp[:, 1]
    A2 = 0.25 * W_proj @ (W_out * a_sym[None, :]) @ W_in
    B3 = W_proj @ (W_out * w1[None, :]) @ W_in + W_proj
    W_d = 0.25 * dwt * W_proj
    C2 = W_proj @ W_mlp2                     # [c, 32]
    bias_out = W_proj @ inputs["b_mlp2"].astype(np.float64)

    bf = ml_dtypes.bfloat16
    return {
        "wb3t": np.ascontiguousarray(B3.T.astype(bf)),
        "wa2t": np.ascontiguousarray(A2.T.astype(bf)),
        "wdt": np.ascontiguousarray(W_d.T.astype(bf)),
        "c2t4": np.ascontiguousarray(np.tile(C2.T.astype(bf), (4, 1))),
        "b1t": np.ascontiguousarray(
            np.tile(inputs["b_mlp1"].astype(f), 4)[:, None]),
        "bout": np.ascontiguousarray(bias_out.astype(f)[:, None]),
        "gb": np.ascontiguousarray(
            np.stack([inputs["bn_gamma"], inputs["bn_beta"]], 1).astype(f)),
    }


def _channel_weights(inputs, xb):
    """Host-exact rank-1 channel weights for one batch sample.

    m = g g^T / max(||g||, eps)^2 with g = mean(x) collapses the channel
    branch into h1 = silu(u (x) w + b1), w = P.x(-1) + Q.x + R.x(+1):
      u = W1 g / max(||g||,eps)^2,  [P Q R] = Wci^T (taps * (Wco g))
    Returns MQ^T/MP^T/MR^T [C, 32] bf16, where M* = u (x) P/Q/R.
    """
    f64 = np.float64
    g = xb.reshape(C, L).astype(f64).mean(axis=1)
    n = max(np.sqrt((g * g).sum()), EPS_NORM)
    ghat = g / n
    v = inputs["w_ch_out"].astype(f64).T @ ghat        # [C] = Wco^T ghat
    pqr = inputs["w_ch_dw"][:, 0, :].astype(f64) * v[:, None]  # [C,3] taps*v
    PQR = inputs["w_ch_in"].astype(f64).T @ pqr        # [C,3] P,Q,R columns
    u = inputs["w_mlp1"].astype(f64) @ ghat            # [32]
    bf = ml_dtypes.bfloat16
    out = {}
    for name, j in [("mpt", 0), ("mqt", 1), ("mrt", 2)]:
        M = np.outer(u, PQR[:, j])                     # [32, C]
        out[name] = np.ascontiguousarray(M.T.astype(bf))   # [C, 32]
    return out


def _make_in_maps(inputs):
    wmap = _fold_weights(inputs)
    x = inputs["x"].astype(np.float32)  # [B, C, H, W]
    in_maps = []
    for b in range(NCORES):
        m = dict(wmap)
        m["x"] = np.ascontiguousarray(
            x[b].reshape(C, L).astype(ml_dtypes.bfloat16))
        m.update(_channel_weights(inputs, x[b]))
        in_maps.append(m)
    return in_maps


def kernel(**inputs):
    from concourse.bass_utils import run_bass_kernel_spmd

    inputs = {k: np.asarray(v) for k, v in inputs.items()}
    if "nc" not in _CACHE:
        _CACHE["nc"] = _build_program()
    nc = _CACHE["nc"]

    in_maps = _make_in_maps(inputs)
    res = run_bass_kernel_spmd(nc, in_maps, list(range(NCORES)))
    out = np.stack([np.asarray(res.results[b]["y"])
                    .astype(np.float32).reshape(C, Himg, Wimg)
                    for b in range(NCORES)])
    return out
